# revision 1
# baseline (speedup 1.0000x reference)
"""Trainium2 Bass kernel for nn_CrossOrganismAttention.

Data-parallel over 8 cores (batch sharded). Per core, rows are processed in
tiles of 128 batch elements (= 384 (b,s) rows). Main activation stream is
feature-major (features on partitions) for matmuls; attention, layernorm
stats and pooling run row-major (batch on partitions), converted via PE
transposes. Matmuls use float32r (1 cy/row at N>=256) for the f32 stream and
bf16 for attention/FFN weights.
"""

import numpy as np

import concourse.bass as bass
import concourse.mybir as mybir
from concourse.bacc import Bacc
from concourse.tile import TileContext
from concourse.bass_utils import run_bass_kernel_spmd

B, S, D, H, DH = 65536, 3, 256, 4, 64
NCORES = 8
NB = 128  # batch elements per tile
EPS = 1e-5
F32 = mybir.dt.float32
F32R = mybir.dt.float32r
BF16 = mybir.dt.bfloat16
AF = mybir.ActivationFunctionType
OP = mybir.AluOpType
AX = mybir.AxisListType
GELU = AF.Gelu  # debug hook: sim lacks Gelu; tests may swap in Tanh


def _chunk_lhst(w_t: np.ndarray) -> np.ndarray:
    """(K, M) lhsT -> (128, K//128 * M) with chunk c at cols [c*M:(c+1)*M]."""
    k, m = w_t.shape
    assert k % 128 == 0
    return np.ascontiguousarray(
        w_t.reshape(k // 128, 128, m).transpose(1, 0, 2).reshape(128, -1)
    )


def _per_part(v: np.ndarray) -> np.ndarray:
    """(n*128,) bias -> (128, n) with chunk c in col c."""
    n = v.shape[0] // 128
    return np.ascontiguousarray(v.reshape(n, 128).T)


# Packed constant layouts: (key, cols). Offsets are cumulative per pack.
_PACK_F32R = [("c_spw2t", 512), ("c_inwt", 1536)]
_PACK_F32 = [("c_spec3", 6), ("c_spb1", 2), ("c_ffn1b", 4), ("c_x2b", 2),
             ("c_g2b", 256), ("c_b2b", 256), ("c_id", 128)]
_PACK_BF16 = [("c_outwt", 512), ("c_ffn1t", 1024), ("c_ffn2t", 1024),
              ("c_g1d", 256), ("c_idb", 128)]
_PACK_SMALL = [("c_w1", 256), ("c_qkvb", 768), ("c_ones", 128)]


def host_consts(p: dict) -> dict:
    """Precompute packed weight/bias constants from the reference params."""
    f = lambda x: np.asarray(x, np.float32)
    sq = 1.0 / np.sqrt(DH)
    in_w, in_b = f(p["in_w"]), f(p["in_b"])
    out_w, out_b = f(p["out_w"]), f(p["out_b"])
    sp_w1, sp_b1 = f(p["sp_w1"]), f(p["sp_b1"])
    sp_w2, sp_b2 = f(p["sp_w2"]), f(p["sp_b2"])
    ffn_w1, ffn_b1 = f(p["ffn_w1"]), f(p["ffn_b1"])
    ffn_w2, ffn_b2 = f(p["ffn_w2"]), f(p["ffn_b2"])
    ln1_g, ln1_b = f(p["ln1_g"]), f(p["ln1_b"])
    ln2_g, ln2_b = f(p["ln2_g"]), f(p["ln2_b"])
    species = f(p["species_emb"])

    # tokens'' = emb + species + sp_b2 + out_b ("species3"); fm layout per chunk.
    sp3 = species + sp_b2[None, :] + out_b[None, :]  # (3, 256)
    spec3 = np.ascontiguousarray(
        sp3.T.reshape(2, 128, 3).transpose(1, 0, 2).reshape(128, 6)
    )  # (128, 2*3): chunk c at cols [3c:3c+3]

    # qkv weights: lhsT = tokens-chunk (stationary), moving rhs = in_w.T.
    inwt = in_w.T.copy()  # (256, 768)
    inwt[:, :D] *= sq
    qkvb = in_b - in_w @ out_b  # compensate out_b folded into tokens''
    qkvb[:D] *= sq

    consts = {
        "c_spec3": spec3,
        "c_w1": np.ascontiguousarray(sp_w1[:, 0][None, :]),  # (1, 256)
        "c_spb1": _per_part(sp_b1),  # (128, 2)
        "c_spw2t": _chunk_lhst(sp_w2.T),  # (128, 2*256)
        "c_inwt": _chunk_lhst(inwt),  # (128, 2*768)
        "c_qkvb": np.ascontiguousarray(qkvb[None, :]),  # (1, 768)
        "c_outwt": _chunk_lhst(out_w.T).astype(np.float32),  # (128, 2*256)
        "c_ffn1t": _chunk_lhst((ffn_w1 * ln1_g[None, :]).T),  # (128, 2*512)
        "c_ffn1b": _per_part(ffn_b1 + ffn_w1 @ ln1_b),  # (128, 4)
        "c_ffn2t": _chunk_lhst(ffn_w2.T),  # (128, 4*256)
        "c_g1d": _chunk_lhst(np.diag(ln1_g))[
            :, [c * 256 + c * 128 + j for c in range(2) for j in range(128)]
        ],  # (128, 2*128): diag block c
        "c_x2b": _per_part(ln1_b + ffn_b2),  # (128, 2)
        "c_g2b": np.ascontiguousarray(np.tile(ln2_g[None, :], (128, 1))),
        "c_b2b": np.ascontiguousarray(np.tile(ln2_b[None, :], (128, 1))),
        "c_id": np.eye(128, dtype=np.float32),
        "c_ones": np.ones((1, 128), np.float32),
    }
    import ml_dtypes

    for k in ("c_outwt", "c_ffn1t", "c_ffn2t", "c_g1d"):
        consts[k] = consts[k].astype(ml_dtypes.bfloat16).view(np.uint16)
    consts["c_idb"] = np.eye(128).astype(ml_dtypes.bfloat16).view(np.uint16)

    def pack(layout):
        return np.ascontiguousarray(
            np.concatenate([consts[k].astype(consts[k].dtype) for k, _ in layout], axis=1)
        )

    return {
        "c_pf32r": pack(_PACK_F32R).astype(np.float32),
        "c_pf32": pack(_PACK_F32).astype(np.float32),
        "c_pbf16": pack(_PACK_BF16),
        "c_small": pack(_PACK_SMALL).astype(np.float32),
    }


def host_inputs(core: int, inputs: dict, consts: dict) -> dict:
    bl = B // NCORES
    b0 = core * bl
    ntiles = bl // NB
    emb = np.ascontiguousarray(np.asarray(inputs["organism_embeddings"], np.float32)[b0 : b0 + bl])
    a = np.asarray(inputs["anomaly_scores"], np.float32)[b0 : b0 + bl]
    # per tile: (s, b) order for fm columns
    a_t = np.ascontiguousarray(
        a.reshape(ntiles, NB, S).transpose(0, 2, 1).reshape(ntiles, S * NB)
    )
    mask = np.asarray(inputs["organism_mask"], bool)[b0 : b0 + bl]
    valid = (~mask).astype(np.float32)
    w = valid / valid.sum(axis=1, keepdims=True)
    wt = np.ascontiguousarray(w.T)  # (3, bl)
    mbt = np.ascontiguousarray((mask.astype(np.float32) * -1e9).T)  # (3, bl)
    m = {"emb": emb, "a": a_t, "wt": wt, "mbt": mbt}
    m.update(consts)
    return m


def build(bl: int, stage=None) -> bass.Bass:
    ntiles = bl // NB
    nc = Bacc()

    EMB = nc.declare_dram_parameter("emb", [bl, S, D], F32, isOutput=False)
    A = nc.declare_dram_parameter("a", [ntiles, S * NB], F32R, isOutput=False)
    WT = nc.declare_dram_parameter("wt", [S, bl], F32, isOutput=False)
    MBT = nc.declare_dram_parameter("mbt", [S, bl], F32, isOutput=False)
    OUT = nc.declare_dram_parameter("out", [bl, D], F32, isOutput=True)

    packs = {
        "c_pf32r": (_PACK_F32R, F32R, 128),
        "c_pf32": (_PACK_F32, F32, 128),
        "c_pbf16": (_PACK_BF16, BF16, 128),
        "c_small": (_PACK_SMALL, F32R, 1),
    }
    cparams = {}
    for pk, (layout, dt, rows) in packs.items():
        w = sum(n for _, n in layout)
        pdt = mybir.dt.uint16 if dt == BF16 else dt
        cparams[pk] = nc.declare_dram_parameter(pk, [rows, w], pdt, isOutput=False)

    with TileContext(nc) as tc:
        with (
            tc.tile_pool(name="consts", bufs=1) as cpool,
            tc.tile_pool(name="pin", bufs=3) as pin,
            tc.tile_pool(name="sb", bufs=2) as sb,
            tc.tile_pool(name="sbs", bufs=3) as sbs,
            tc.tile_pool(name="pout", bufs=3) as pout,
            tc.tile_pool(name="psA", bufs=4, space="PSUM") as psA,
            tc.tile_pool(name="psQ", bufs=2, space="PSUM") as psQ,
            tc.tile_pool(name="psB", bufs=2, space="PSUM") as psB,
        ):
            C = {}
            for pk, (layout, dt, rows) in packs.items():
                w = sum(n for _, n in layout)
                ct = cpool.tile([rows, w], dt, tag=pk)
                src = cparams[pk][:, :]
                if dt == BF16:
                    src = src.bitcast(BF16)
                nc.sync.dma_start(ct[:], src)
                off = 0
                for k, n in layout:
                    C[k] = ct[:, off : off + n]
                    off += n
            # PE warm-up: touch each const pack once on PE so no later
            # matmul needs more than one fresh semaphore wait (the LW
            # struct has a single wait slot).
            wps = psA.tile([128, 384], F32, tag="psA")
            nc.tensor.transpose(wps[:, 0:128], C["c_id"], C["c_id"])
            nc.tensor.matmul(wps[:, 0:128],
                             C["c_spw2t"][:, 0:128], C["c_spw2t"][:, 0:128])
            nc.tensor.matmul(wps[:, 0:256], C["c_ones"], C["c_w1"])
            wpb = psB.tile([128, 384], BF16, tag="psB")
            nc.tensor.transpose(wpb[:, 0:128], C["c_idb"], C["c_idb"])
            # DVE/ACT warm-up: observe each const-pack DMA semaphore once.
            wsc = sbs.tile([128, 8], F32, tag="wsc")
            nc.vector.tensor_copy(wsc[:, 0:1], C["c_id"][:, 0:1])
            nc.vector.tensor_copy(wsc[:, 1:2], C["c_idb"][:, 0:1])
            nc.vector.tensor_copy(wsc[:, 2:3], C["c_spw2t"][:, 0:1])
            nc.vector.tensor_copy(wsc[0:1, 3:4], C["c_w1"][0:1, 0:1])
            nc.scalar.activation(wsc[:, 4:5], C["c_id"][:, 1:2], AF.Copy)
            nc.scalar.activation(wsc[:, 5:6], C["c_idb"][:, 1:2], AF.Copy)
            nc.scalar.activation(wsc[:, 6:7], C["c_spw2t"][:, 1:2], AF.Copy)
            nc.scalar.activation(wsc[0:1, 7:8], C["c_w1"][0:1, 1:2], AF.Copy)
            for t in range(ntiles):
                _tile_body(nc, tc, C, pin, sb, sbs, pout, psA, psQ, psB,
                           EMB, A, WT, MBT, OUT, t, stage)
    nc.compile()
    return nc


def _dbg(nc, OUT, B0, ap):
    nc.sync.dma_start(OUT[B0 : B0 + NB, :], ap)


def _tile_body(nc, tc, C, pin, sb, sbs, pout, psA, psQ, psB,
               EMB, A, WT, MBT, OUT, t, stage=None):
    B0 = t * NB
    mm = nc.tensor.matmul

    # ---- input DMAs ----
    emb_rm = pin.tile([128, 768], F32, tag="emb_rm")
    nc.sync.dma_start(emb_rm[:].rearrange("p (s d) -> p s d", s=3), EMB[B0 : B0 + NB])
    a_t = pin.tile([1, 384], F32R, tag="a_t")
    nc.sync.dma_start(a_t[:], A[t : t + 1, :])
    w_t = pin.tile([3, 128], F32, tag="w_t")
    nc.sync.dma_start(w_t[:], WT[:, B0 : B0 + NB])
    mb_t = pin.tile([3, 128], F32, tag="mb_t")
    nc.sync.dma_start(mb_t[:], MBT[:, B0 : B0 + NB])

    idf = C["c_id"]
    idb = C["c_idb"]

    # ---- emb -> fm (+species/sp_b2/out_b) ----
    emb2 = sb.tile([128, 768], BF16, tag="emb2")  # (c, s*128+b)
    for c in range(2):
        ps = psA.tile([128, 384], F32, tag="psA")
        for s in range(3):
            nc.tensor.transpose(
                ps[:, s * 128 : s * 128 + 128],
                emb_rm[:, s * 256 + c * 128 : s * 256 + c * 128 + 128],
                idf,
            )
        nc.vector.tensor_tensor(
            out=emb2[:, c * 384 : (c + 1) * 384].rearrange("p (s b) -> p s b", s=3),
            in0=ps[:].rearrange("p (s b) -> p s b", s=3),
            in1=C["c_spec3"][:, c * 3 : c * 3 + 3][:, :, None].broadcast_to((128, 3, 128)),
            op=OP.add,
        )

    if stage == 1:
        return _dbg(nc, OUT, B0, emb2[:, 0:512].bitcast(F32))

    # ---- score MLP: h = gelu(a * w1 + b1) ----
    h2 = sb.tile([128, 768], F32R, tag="h2")
    for c in range(2):
        ps = psA.tile([128, 384], F32, tag="psA")
        mm(ps[:], C["c_w1"][0:1, c * 128 : c * 128 + 128],
           a_t[:])
        nc.scalar.activation(h2[:, c * 384 : (c + 1) * 384], ps[:], GELU,
                             bias=C["c_spb1"][:, c : c + 1])

    if stage == 2:
        return _dbg(nc, OUT, B0, h2[:, 0:256].bitcast(F32))

    # ---- tokens = emb2 + sp_w2 @ h ----
    tok2 = sb.tile([128, 768], F32R, tag="tok2")
    for c in range(2):
        ps = psA.tile([128, 384], F32, tag="psA")
        for kc in range(2):
            mm(ps[:],
               C["c_spw2t"][:, kc * 256 + c * 128 : kc * 256 + c * 128 + 128],
               h2[:, kc * 384 : (kc + 1) * 384],
               start=(kc == 0), stop=(kc == 1))
        nc.vector.tensor_tensor(out=tok2[:, c * 384 : (c + 1) * 384],
                                in0=ps[:], in1=emb2[:, c * 384 : (c + 1) * 384],
                                op=OP.add)

    if stage == 3:
        return _dbg(nc, OUT, B0, tok2[:, 0:256].bitcast(F32))

    # ---- qkv (row-major out, per s) ----
    qkv3 = sb.tile([128, 2304], BF16, tag="qkv3")  # (s, 768)
    for s in range(3):
        for nh in range(2):
            ps = psA.tile([128, 384], F32, tag="psA")
            for kc in range(2):
                mm(ps[:],
                   tok2[:, kc * 384 + s * 128 : kc * 384 + s * 128 + 128],
                   C["c_inwt"][:, kc * 768 + nh * 384 : kc * 768 + (nh + 1) * 384],
                   start=(kc == 0), stop=False)
            mm(ps[:],
               C["c_ones"],
               C["c_qkvb"][0:1, nh * 384 : (nh + 1) * 384],
               start=False, stop=True)
            nc.scalar.activation(
                qkv3[:, s * 768 + nh * 384 : s * 768 + (nh + 1) * 384],
                ps[:], AF.Copy)

    if stage == 4:
        return _dbg(nc, OUT, B0, qkv3[:, 0:512].bitcast(F32))

    # ---- mask columns -> row-major ----
    psm = psA.tile([128, 384], F32, tag="psA")
    nc.tensor.transpose(psm[:, 0:3], w_t[:], idf[0:3, 0:3])
    nc.tensor.transpose(psm[:, 3:6], mb_t[:], idf[0:3, 0:3])
    wm = sbs.tile([128, 6], F32, tag="wm")
    nc.vector.tensor_copy(wm[:], psm[:, 0:6])

    if stage == 5:
        return _dbg(nc, OUT, B0, emb_rm[:, 0:256])

    # ---- attention scores ----
    qv = qkv3[:].rearrange("p (s f) -> p s f", s=3)
    prod = sb.tile([128, 2304], BF16, tag="prod")
    nc.vector.tensor_tensor(
        out=prod[:].rearrange("p (q k f) -> p q k f", q=3, k=3),
        in0=qv[:, :, None, 0:256].broadcast_to((128, 3, 3, 256)),
        in1=qv[:, None, :, 256:512].broadcast_to((128, 3, 3, 256)),
        op=OP.mult,
    )
    att = sbs.tile([128, 36], BF16, tag="att")  # (q, k, h)
    with nc.allow_low_precision("bf16 attention scores"):
        nc.vector.tensor_reduce(
            out=att[:].rearrange("p (q k h) -> p q k h", q=3, k=3),
            in_=prod[:].rearrange("p (q k h e) -> p q k h e", q=3, k=3, h=4),
            axis=AX.X, op=OP.add,
        )
    attv = att[:].rearrange("p (q k h) -> p q k h", q=3, k=3)
    attm = sbs.tile([128, 36], F32, tag="attm")
    amv = attm[:].rearrange("p (q k h) -> p q k h", q=3, k=3)
    nc.vector.tensor_tensor(
        out=amv, in0=attv,
        in1=wm[:, 3:6][:, None, :, None].broadcast_to((128, 3, 3, 4)),
        op=OP.add,
    )
    mx = sbs.tile([128, 12], F32, tag="mx")
    mxv = mx[:].rearrange("p (q h) -> p q h", q=3)
    nc.vector.tensor_tensor(out=mxv, in0=amv[:, :, 0], in1=amv[:, :, 1], op=OP.max)
    nc.vector.tensor_tensor(out=mxv, in0=mxv, in1=amv[:, :, 2], op=OP.max)
    es = sbs.tile([128, 36], F32, tag="es")
    esv = es[:].rearrange("p (q k h) -> p q k h", q=3, k=3)
    nc.vector.tensor_tensor(
        out=esv, in0=amv,
        in1=mxv[:, :, None, :].broadcast_to((128, 3, 3, 4)), op=OP.subtract,
    )
    nc.scalar.activation(es[:], es[:], AF.Exp)
    den = sbs.tile([128, 12], F32, tag="den")
    dv = den[:].rearrange("p (q h) -> p q h", q=3)
    nc.vector.tensor_tensor(out=dv, in0=esv[:, :, 0], in1=esv[:, :, 1], op=OP.add)
    nc.vector.tensor_tensor(out=dv, in0=dv, in1=esv[:, :, 2], op=OP.add)
    rden = sbs.tile([128, 12], F32, tag="rden")
    nc.vector.reciprocal(rden[:], den[:])
    p = sbs.tile([128, 36], F32, tag="p")
    pv = p[:].rearrange("p (q k h) -> p q k h", q=3, k=3)
    nc.vector.tensor_tensor(
        out=pv, in0=esv,
        in1=rden[:].rearrange("p (q h) -> p q h", q=3)[:, :, None, :].broadcast_to((128, 3, 3, 4)),
        op=OP.mult,
    )

    if stage == 6:
        return _dbg(nc, OUT, B0, prod[:, 0:512].bitcast(F32))

    # ---- mix: o_q = v0 + p1*(v1-v0) + p2*(v2-v0) ----
    d1 = sbs.tile([128, 256], BF16, tag="d1")
    d2 = sbs.tile([128, 256], BF16, tag="d2")
    nc.vector.tensor_tensor(out=d1[:], in0=qv[:, 1, 512:768], in1=qv[:, 0, 512:768], op=OP.subtract)
    nc.vector.tensor_tensor(out=d2[:], in0=qv[:, 2, 512:768], in1=qv[:, 0, 512:768], op=OP.subtract)
    t1 = sb.tile([128, 768], BF16, tag="t1")
    t2 = sb.tile([128, 768], BF16, tag="t2")
    for dd, tt_, k in ((d1, t1, 1), (d2, t2, 2)):
        nc.vector.tensor_tensor(
            out=tt_[:].rearrange("p (q h e) -> p q h e", q=3, h=4),
            in0=dd[:, None, :].broadcast_to((128, 3, 256)).rearrange("p q (h e) -> p q h e", h=4),
            in1=pv[:, :, k, :, None].broadcast_to((128, 3, 4, 64)),
            op=OP.mult,
        )
    o = sb.tile([128, 768], BF16, tag="o")
    nc.vector.tensor_tensor(out=o[:], in0=t1[:], in1=t2[:], op=OP.add)
    nc.vector.tensor_tensor(
        out=o[:].rearrange("p (q f) -> p q f", q=3),
        in0=o[:].rearrange("p (q f) -> p q f", q=3),
        in1=qv[:, 0:1, 512:768].broadcast_to((128, 3, 256)),
        op=OP.add,
    )

    if stage == 7:
        return _dbg(nc, OUT, B0, o[:, 0:512].bitcast(F32))

    # ---- o -> fm; out-proj; x1 = tokens'' + o @ out_w.T ----
    ofm = sb.tile([128, 768], BF16, tag="ofm")
    for c in range(2):
        ps = psB.tile([128, 384], BF16, tag="psB")
        for q in range(3):
            nc.tensor.transpose(
                ps[:, q * 128 : q * 128 + 128],
                o[:, q * 256 + c * 128 : q * 256 + c * 128 + 128],
                idb,
            )
        nc.vector.tensor_copy(ofm[:, c * 384 : (c + 1) * 384], ps[:])
    x1f = sb.tile([128, 768], BF16, tag="x1f")
    for c in range(2):
        ps = psA.tile([128, 384], F32, tag="psA")
        for kc in range(2):
            mm(ps[:],
               C["c_outwt"][:, kc * 256 + c * 128 : kc * 256 + c * 128 + 128],
               ofm[:, kc * 384 : (kc + 1) * 384],
               start=(kc == 0), stop=(kc == 1))
        nc.vector.tensor_tensor(out=x1f[:, c * 384 : (c + 1) * 384], in0=ps[:],
                                in1=tok2[:, c * 384 : (c + 1) * 384], op=OP.add)

    if stage == 8:
        return _dbg(nc, OUT, B0, x1f[:, 0:512].bitcast(F32))

    # ---- LN1 (row-major) ----
    n1rm = _layernorm_rm(nc, tc, sb, sbs, psQ, psA, x1f, idb, "1")

    if stage == 9:
        return _dbg(nc, OUT, B0, n1rm[:, 0:512].bitcast(F32))

    # ---- n1 -> fm ----
    n1f = sb.tile([128, 768], BF16, tag="n1f")
    for c in range(2):
        ps = psB.tile([128, 384], BF16, tag="psB")
        for s in range(3):
            nc.tensor.transpose(
                ps[:, s * 128 : s * 128 + 128],
                n1rm[:, s * 256 + c * 128 : s * 256 + c * 128 + 128],
                idb,
            )
        nc.vector.tensor_copy(n1f[:, c * 384 : (c + 1) * 384], ps[:])

    if stage == 10:
        return _dbg(nc, OUT, B0, n1f[:, 0:512].bitcast(F32))

    # ---- FFN ----
    f1 = sb.tile([128, 1536], BF16, tag="f1")
    for oc in range(4):
        ps = psA.tile([128, 384], F32, tag="psA")
        for kc in range(2):
            mm(ps[:],
               C["c_ffn1t"][:, kc * 512 + oc * 128 : kc * 512 + oc * 128 + 128],
               n1f[:, kc * 384 : (kc + 1) * 384],
               start=(kc == 0), stop=(kc == 1))
        nc.scalar.activation(f1[:, oc * 384 : (oc + 1) * 384], ps[:], GELU,
                             bias=C["c_ffn1b"][:, oc : oc + 1])
    x2f = sb.tile([128, 768], BF16, tag="x2f")
    for c in range(2):
        ps = psA.tile([128, 384], F32, tag="psA")
        for kc in range(4):
            mm(ps[:],
               C["c_ffn2t"][:, kc * 256 + c * 128 : kc * 256 + c * 128 + 128],
               f1[:, kc * 384 : (kc + 1) * 384],
               start=(kc == 0), stop=False)
        mm(ps[:], C["c_g1d"][:, c * 128 : (c + 1) * 128],
           n1f[:, c * 384 : (c + 1) * 384], start=False, stop=True)
        nc.scalar.activation(x2f[:, c * 384 : (c + 1) * 384], ps[:], AF.Identity,
                             bias=C["c_x2b"][:, c : c + 1])

    if stage == 11:
        return _dbg(nc, OUT, B0, x2f[:, 0:512].bitcast(F32))

    # ---- LN2 (row-major) ----
    n2rm = _layernorm_rm(nc, tc, sb, sbs, psQ, psA, x2f, idb, "2")

    if stage == 12:
        return _dbg(nc, OUT, B0, n2rm[:, 0:512].bitcast(F32))

    # ---- masked mean pool + ln2 affine ----
    acc = pout.tile([128, 256], F32, tag="acc")
    nc.vector.scalar_tensor_tensor(
        out=acc[:], in0=n2rm[:, 0:256], scalar=wm[:, 0:1], in1=n2rm[:, 0:256],
        op0=OP.mult, op1=OP.bypass,
    )
    for s in (1, 2):
        nc.vector.scalar_tensor_tensor(
            out=acc[:], in0=n2rm[:, s * 256 : (s + 1) * 256],
            scalar=wm[:, s : s + 1], in1=acc[:], op0=OP.mult, op1=OP.add,
        )
    outt = pout.tile([128, 256], F32, tag="outt")
    nc.vector.tensor_tensor(out=outt[:], in0=acc[:], in1=C["c_g2b"], op=OP.mult)
    nc.vector.tensor_tensor(out=outt[:], in0=outt[:], in1=C["c_b2b"], op=OP.add)
    nc.sync.dma_start(OUT[B0 : B0 + NB, :], outt[:])


LN_STAGE = None


def _layernorm_rm(nc, tc, sb, sbs, psQ, psA, xf, idb, suffix):
    """xf: (128, 2*384) bf16 feature-major -> normalized row-major (128, 768)
    bf16 (no affine)."""
    psr = psQ.tile([128, 768], BF16, tag="psQ")
    for c in range(2):
        for s in range(3):
            nc.tensor.transpose(
                psr[:, s * 256 + c * 128 : s * 256 + c * 128 + 128],
                xf[:, c * 384 + s * 128 : c * 384 + s * 128 + 128],
                idb,
            )
    xrm = sb.tile([128, 768], BF16, tag="xrm" + suffix)
    sums = sbs.tile([128, 3], F32, tag="sums" + suffix)
    sqs = sbs.tile([128, 3], F32, tag="sqs" + suffix)
    scr = sb.tile([128, 768], BF16, tag="scr" + suffix)
    for s in range(3):
        nc.scalar.activation(
            xrm[:, s * 256 : (s + 1) * 256], psr[:, s * 256 : (s + 1) * 256],
            AF.Copy, accum_out=sums[:, s : s + 1],
        )
        nc.scalar.activation(
            scr[:, s * 256 : (s + 1) * 256], xrm[:, s * 256 : (s + 1) * 256],
            AF.Square, accum_out=sqs[:, s : s + 1],
        )
    if LN_STAGE in (81, 815, 82):
        return xrm
    mean = sbs.tile([128, 3], F32, tag="mean" + suffix)
    nc.vector.tensor_scalar_mul(mean[:], sums[:], 1.0 / 256.0)
    ex2 = sbs.tile([128, 3], F32, tag="ex2" + suffix)
    nc.vector.tensor_scalar(out=ex2[:], in0=sqs[:], scalar1=1.0 / 256.0,
                            scalar2=EPS, op0=OP.mult, op1=OP.add)
    m2 = sbs.tile([128, 3], F32, tag="m2" + suffix)
    nc.vector.tensor_tensor(out=m2[:], in0=mean[:], in1=mean[:], op=OP.mult)
    var = sbs.tile([128, 3], F32, tag="var" + suffix)
    nc.vector.tensor_tensor(out=var[:], in0=ex2[:], in1=m2[:], op=OP.subtract)
    sd = sbs.tile([128, 3], F32, tag="sd" + suffix)
    nc.scalar.activation(sd[:], var[:], AF.Sqrt)
    rstd = sbs.tile([128, 3], F32, tag="rstd" + suffix)
    nc.vector.reciprocal(rstd[:], sd[:])
    mb = sbs.tile([128, 3], F32, tag="mb" + suffix)
    nc.vector.scalar_tensor_tensor(out=mb[:], in0=mean[:], scalar=-1.0,
                                   in1=rstd[:], op0=OP.mult, op1=OP.mult)
    if LN_STAGE == 83:
        return xrm
    nrm = sb.tile([128, 768], BF16, tag="nrm" + suffix)
    for s in range(3):
        nc.scalar.activation(
            nrm[:, s * 256 : (s + 1) * 256], xrm[:, s * 256 : (s + 1) * 256],
            AF.Identity, bias=mb[:, s : s + 1], scale=rstd[:, s : s + 1],
        )
    return nrm


_CACHE: dict = {}


def _get_nc(bl: int) -> bass.Bass:
    if bl not in _CACHE:
        _CACHE[bl] = build(bl)
    return _CACHE[bl]


LAST_RESULT = None


def kernel(**inputs) -> np.ndarray:
    global LAST_RESULT
    consts = host_consts(inputs)
    nc = _get_nc(B // NCORES)
    in_maps = [host_inputs(i, inputs, consts) for i in range(NCORES)]
    res = run_bass_kernel_spmd(nc, in_maps, core_ids=list(range(NCORES)))
    LAST_RESULT = res
    return np.concatenate([r["out"] for r in res.results], axis=0)



# revision 23
# speedup vs baseline: 1.5451x; 1.5451x over previous
"""Trainium2 Bass kernel for nn_CrossOrganismAttention (v2).

Data-parallel over 8 cores (batch sharded). Per core, rows processed in
supertiles of NB=256 batch rows (= 768 (s,b) columns feature-major, or
2 row-major blocks of 128). Feature-major main stream; attention/softmax
row-major. qkv/out biases folded via softmax invariance (only a 4-col
"vk" weight extension survives); species/residual/bias adds ride on
vector drains; LN stats via PE ones-matmuls; rstd via Exp(-0.5*Ln(var+eps))
so all non-Gelu ACT work stays in one table set; LN2 fused into the
masked-mean pool.
"""

import numpy as np

import concourse.bass as bass
import concourse.mybir as mybir
from concourse.bacc import Bacc
from concourse.tile import TileContext
from concourse.bass_utils import run_bass_kernel_spmd

B, S, D, H, DH = 65536, 3, 256, 4, 64
NCORES = 8
NB = 256  # batch rows per supertile (2 row-major blocks)
W = 768  # feature-major column count per chunk: (s, b) = 3*256
EPS = 1e-5
F32 = mybir.dt.float32
F32R = mybir.dt.float32r
BF16 = mybir.dt.bfloat16
AF = mybir.ActivationFunctionType
OP = mybir.AluOpType
AX = mybir.AxisListType
GELU = AF.Gelu


def _chunk_lhst(w_t: np.ndarray) -> np.ndarray:
    """(K, M) lhsT -> (128, K//128 * M) with chunk c at cols [c*M:(c+1)*M]."""
    k, m = w_t.shape
    assert k % 128 == 0
    return np.ascontiguousarray(
        w_t.reshape(k // 128, 128, m).transpose(1, 0, 2).reshape(128, -1)
    )


def _per_part(v: np.ndarray) -> np.ndarray:
    """(n*128,) vector -> (128, n) with chunk c in col c."""
    n = v.shape[0] // 128
    return np.ascontiguousarray(v.reshape(n, 128).T)


# Packed constant layouts: (key, cols).
_PACK_BF16 = [("c_qkvw", 2 * 772), ("c_spw2t", 512), ("c_outwt", 512),
              ("c_ffn1t", 1024), ("c_ffn2t", 1024), ("c_idb", 128),
              ("c_recipD", 1)]
_PACK_F32 = [("c_id", 128), ("c_spec", 6), ("c_spb1", 2), ("c_ffn1b", 4),
             ("c_x2b", 2), ("c_attC", 2), ("c_g1pp", 2), ("c_g2pp", 2)]
_PACK_SMALL = [("c_w1row", 256), ("c_ones1", 128), ("c_onesrow", 768),
               ("c_g1row", 256), ("c_ng2row", 256), ("c_b2row", 256),
               ("c_eps", 1)]


def host_consts(p: dict) -> dict:
    f = lambda x: np.asarray(x, np.float32)
    sq = 1.0 / np.sqrt(DH)
    in_w, in_b = f(p["in_w"]), f(p["in_b"])
    out_w, out_b = f(p["out_w"]), f(p["out_b"])
    sp_w1, sp_b1 = f(p["sp_w1"]), f(p["sp_b1"])
    sp_w2, sp_b2 = f(p["sp_w2"]), f(p["sp_b2"])
    ffn_w1, ffn_b1 = f(p["ffn_w1"]), f(p["ffn_b1"])
    ffn_w2, ffn_b2 = f(p["ffn_w2"]), f(p["ffn_b2"])
    ln1_g, ln1_b = f(p["ln1_g"]), f(p["ln1_b"])
    ln2_g, ln2_b = f(p["ln2_g"]), f(p["ln2_b"])
    species = f(p["species_emb"])

    wq = in_w[0:256] * sq
    bq = in_b[0:256] * sq
    wk = in_w[256:512]
    wv = in_w[512:768]
    bv = in_b[512:768]
    # vk[d, h] = wk_h^T @ bq_h  (the only qkv-bias term softmax keeps)
    wvk = np.zeros((256, 4), np.float32)
    for h in range(H):
        wvk[:, h] = wk[h * DH:(h + 1) * DH].T @ bq[h * DH:(h + 1) * DH]
    # qkv weights (256, 772): [q 0:256 | k 256:512 | vk 512:516 | v 516:772]
    wqkv = np.concatenate([wq.T, wk.T, wvk, wv.T], axis=1)
    attC = out_w @ bv + out_b  # x1 = o0 @ Wo^T + attC + tok

    consts = {
        "c_qkvw": _chunk_lhst(wqkv),                      # (128, 2*772)
        "c_spw2t": _chunk_lhst(sp_w2.T),                  # (128, 2*256)
        "c_outwt": _chunk_lhst(out_w.T),                  # (128, 2*256)
        "c_ffn1t": _chunk_lhst(ffn_w1.T),                 # (128, 2*512)
        "c_ffn2t": _chunk_lhst(ffn_w2.T),                 # (128, 4*256)
        "c_idb": np.eye(128, dtype=np.float32),
        "c_recipD": np.full((128, 1), 1.0 / 256.0, np.float32),
        "c_id": np.eye(128, dtype=np.float32),
        "c_spec": np.ascontiguousarray(
            (species + sp_b2[None, :]).T.reshape(2, 128, 3)
            .transpose(1, 0, 2).reshape(128, 6)),          # (128, (c,s))
        "c_spb1": _per_part(sp_b1),
        "c_ffn1b": _per_part(ffn_b1 + ffn_w1 @ ln1_b),    # ln1_b folded
        "c_x2b": _per_part(ffn_b2 + ln1_b),
        "c_attC": _per_part(attC),
        "c_g1pp": _per_part(ln1_g),
        "c_g2pp": _per_part(ln2_g),
        "c_w1row": np.ascontiguousarray(sp_w1[:, 0][None, :]),   # (1, 256)
        "c_ones1": np.ones((1, 128), np.float32),
        "c_onesrow": np.ones((1, 768), np.float32),
        "c_g1row": np.ascontiguousarray(ln1_g[None, :]),  # (1, 256)
        "c_ng2row": np.ascontiguousarray(-ln2_g[None, :]),
        "c_b2row": np.ascontiguousarray(ln2_b[None, :]),
        "c_eps": np.full((1, 1), EPS, np.float32),
    }
    import ml_dtypes
    for k, _ in _PACK_BF16:
        consts[k] = consts[k].astype(ml_dtypes.bfloat16).view(np.uint16)

    def pack(layout):
        return np.ascontiguousarray(
            np.concatenate([consts[k] for k, _ in layout], axis=1))

    return {
        "c_pbf16": pack(_PACK_BF16),
        "c_pf32": pack(_PACK_F32).astype(np.float32),
        "c_small": pack(_PACK_SMALL).astype(np.float32),
    }


def host_inputs(core: int, inputs: dict, consts: dict) -> dict:
    bl = B // NCORES
    b0 = core * bl
    nst = bl // NB
    emb = np.ascontiguousarray(
        np.asarray(inputs["organism_embeddings"], np.float32)[b0:b0 + bl])
    a = np.asarray(inputs["anomaly_scores"], np.float32)[b0:b0 + bl]
    a_t = np.ascontiguousarray(
        a.reshape(nst, NB, S).transpose(0, 2, 1).reshape(nst, S * NB))
    mask = np.asarray(inputs["organism_mask"], bool)[b0:b0 + bl]
    valid = (~mask).astype(np.float32)
    w = valid / valid.sum(axis=1, keepdims=True)
    wrow = np.ascontiguousarray(
        w.reshape(nst, NB, S).transpose(0, 2, 1).reshape(nst, S * NB))
    mb = (mask.astype(np.float32) * -1e9)
    mbt = np.ascontiguousarray(
        mb.reshape(nst, NB, S).transpose(0, 2, 1))  # (nst, 3, 256)
    m = {"emb": emb, "a": a_t, "wr": wrow, "mbt": mbt}
    m.update(consts)
    return m


def build(bl: int, stage=None) -> bass.Bass:
    nst = bl // NB
    nc = Bacc()

    EMB = nc.declare_dram_parameter("emb", [bl, S, D], F32, isOutput=False)
    A = nc.declare_dram_parameter("a", [nst, S * NB], F32R, isOutput=False)
    WR = nc.declare_dram_parameter("wr", [nst, S * NB], F32, isOutput=False)
    MBT = nc.declare_dram_parameter("mbt", [nst, S, NB], F32, isOutput=False)
    OUT = nc.declare_dram_parameter("out", [bl, D], F32, isOutput=True)

    packs = {
        "c_pbf16": (_PACK_BF16, BF16, 128),
        "c_pf32": (_PACK_F32, F32, 128),
        "c_small": (_PACK_SMALL, F32R, 1),
    }
    cparams = {}
    for pk, (layout, dt, rows) in packs.items():
        wdt = sum(n for _, n in layout)
        pdt = mybir.dt.uint16 if dt == BF16 else dt
        cparams[pk] = nc.declare_dram_parameter(pk, [rows, wdt], pdt, isOutput=False)

    with TileContext(nc) as tc:
        with (
            tc.tile_pool(name="consts", bufs=1) as cpool,
            tc.tile_pool(name="pin", bufs=3) as pin,
            tc.tile_pool(name="pin2", bufs=2) as pin2,
            tc.tile_pool(name="sb", bufs=2) as sb,
            tc.tile_pool(name="sb3", bufs=3) as sb3,
            tc.tile_pool(name="sbs", bufs=3) as sbs,
            tc.tile_pool(name="pstat", bufs=2) as pstat,
            tc.tile_pool(name="pout", bufs=3) as pout,
            tc.tile_pool(name="ps", bufs=3, space="PSUM") as ps,
            tc.tile_pool(name="psb", bufs=2, space="PSUM") as psb,
        ):
            C = {}
            for pk, (layout, dt, rows) in packs.items():
                wdt = sum(n for _, n in layout)
                ct = cpool.tile([rows, wdt], dt, tag=pk)
                src = cparams[pk][:, :]
                if dt == BF16:
                    src = src.bitcast(BF16)
                nc.sync.dma_start(ct[:], src)
                off = 0
                for k, n in layout:
                    C[k] = ct[:, off:off + n]
                    off += n
            _warmup(nc, C, ps, psb, sbs)
            pools = dict(pin=pin, pin2=pin2, sb=sb, sb3=sb3, sbs=sbs,
                         pstat=pstat, pout=pout, ps=ps, psb=psb)
            h_next = _h_phase(nc, C, pools, A, 0)
            st_prev = None
            for t in range(nst + 1):
                st = None
                if t < nst:
                    st = _front(nc, C, pools, EMB, WR, MBT, t, h_next)
                if st_prev is not None:
                    _back_a(nc, C, pools, t - 1, st_prev)
                if t + 1 < nst:
                    h_next = _h_phase(nc, C, pools, A, t + 1)
                if st_prev is not None:
                    _back_b(nc, C, pools, OUT, t - 1, st_prev)
                st_prev = st
    nc.compile()
    return nc


def _warmup(nc, C, ps, psb, sbs):
    """Touch each const pack once on PE/DVE/ACT so later ops have at most
    one fresh semaphore wait."""
    wps = ps.tile([128, W], F32, tag="psA")
    nc.tensor.transpose(wps[:, 0:128], C["c_id"], C["c_id"])
    nc.tensor.matmul(wps[:, 0:128], C["c_qkvw"][:, 0:128], C["c_idb"])
    nc.tensor.matmul(wps[:, 0:384], C["c_ones1"], C["c_onesrow"][0:1, 0:384])
    wpb = psb.tile([128, W], BF16, tag="psB")
    nc.tensor.transpose(wpb[:, 0:128], C["c_idb"], C["c_idb"])
    wsc = sbs.tile([128, 8], F32, tag="wsc")
    nc.vector.tensor_copy(wsc[:, 0:1], C["c_id"][:, 0:1])
    nc.vector.tensor_copy(wsc[:, 1:2], C["c_idb"][:, 0:1])
    nc.vector.tensor_copy(wsc[0:1, 2:3], C["c_w1row"][0:1, 0:1])
    nc.scalar.activation(wsc[:, 4:5], C["c_id"][:, 1:2], AF.Copy)
    nc.scalar.activation(wsc[:, 5:6], C["c_idb"][:, 1:2], AF.Copy)
    nc.scalar.activation(wsc[0:1, 6:7], C["c_w1row"][0:1, 1:2], AF.Copy)


def _dbg(nc, OUT, B0, ap):
    nc.sync.dma_start(OUT[B0:B0 + 128, :], ap)


def _ln_sums(nc, C, ps, mm, x_sb, xsq):
    """psum [2, W]: row0 = mean(x), row1 = E[x^2]  (1/256 folded in lhsT)."""
    sums = ps.tile([33, W], F32, tag="psA")  # row 0 = mean, row 32 = E[x^2]
    for bk in range(2):
        sl = slice(bk * 384, (bk + 1) * 384)
        for c in range(2):
            mm(sums[0:1, sl], C["c_recipD"], x_sb[c][:, sl],
               start=(c == 0), stop=(c == 1))
        for c in range(2):
            mm(sums[32:33, sl], C["c_recipD"], xsq[c][:, sl],
               start=(c == 0), stop=(c == 1), skip_group_check=True)
    return sums


def _ln_stats(nc, C, pstat, sums, sfx):
    """rstd from sums via in-place chain; returns (rstd, sta_scratch)."""
    sta = pstat.tile([1, W], F32, tag="sta_" + sfx)
    nc.scalar.activation(sta[:], sums[0:1, :], AF.Square)
    nc.vector.tensor_tensor(out=sta[:], in0=sums[32:33, :], in1=sta[:],
                            op=OP.subtract)
    nc.scalar.activation(sta[:], sta[:], AF.Ln, bias=C["c_eps"][0:1, 0:1])
    rstd = pstat.tile([1, W], F32R, tag="stb_" + sfx)
    nc.scalar.activation(rstd[:], sta[:], AF.Exp, scale=-0.5)
    return rstd, sta


def _ln_apply(nc, C, sb, sb3, pstat, ps, mm, x_sb, xsq, sfx):
    """x1na_c = g1_c * ((x_c - m) * rstd) + 0  (ln1_b folded downstream)."""
    sums = _ln_sums(nc, C, ps, mm, x_sb, xsq)
    rstd, sta = _ln_stats(nc, C, pstat, sums, sfx)
    negmr = pstat.tile([1, W], F32R, tag="stc_" + sfx)
    nc.vector.scalar_tensor_tensor(out=negmr[:], in0=sums[0:1, :], scalar=-1.0,
                                   in1=rstd[:], op0=OP.mult, op1=OP.mult)
    rp = ps.tile([128, W], F32, tag="psA")
    for bk in range(2):
        sl = slice(bk * 384, (bk + 1) * 384)
        mm(rp[:, sl], C["c_ones1"], rstd[0:1, sl])
    out = []
    for c in range(2):
        wp = ps.tile([128, W], F32, tag="psA")
        for bk in range(2):
            sl = slice(bk * 384, (bk + 1) * 384)
            mm(wp[:, sl], C["c_g1row"][0:1, c * 128:(c + 1) * 128],
               negmr[0:1, sl])
        tt = sb.tile([128, W], BF16, tag=f"lnt{sfx}{c}")
        nc.vector.tensor_tensor(out=tt[:], in0=x_sb[c][:], in1=rp[:], op=OP.mult)
        xa = sb.tile([128, W], BF16, tag=f"xna{sfx}{c}")
        nc.vector.scalar_tensor_tensor(
            out=xa[:], in0=tt[:], scalar=C["c_g1pp"][:, c:c + 1],
            in1=wp[:], op0=OP.mult, op1=OP.add)
        out.append(xa)
    return out


def _h_phase(nc, C, pools, A, t):
    """score-MLP for supertile t: h = gelu(w1 x a + b1), fm per chunk."""
    pin2, sb, ps = pools["pin2"], pools["sb"], pools["ps"]
    mm = nc.tensor.matmul
    a_row = pin2.tile([1, W], F32R, tag="a_row")
    nc.sync.dma_start(a_row[:], A[t:t + 1, :])
    h_sb = []
    for c in range(2):
        hp = ps.tile([128, W], F32, tag="psA")
        for (lo, hi) in PIECES:
            mm(hp[:, lo:hi],
               C["c_w1row"][0:1, c * 128:(c + 1) * 128],
               a_row[0:1, lo:hi])
        h = sb.tile([128, W], BF16, tag=f"h{c}")
        nc.scalar.activation(h[:], hp[:], GELU, bias=C["c_spb1"][:, c:c + 1])
        h_sb.append(h)
    return h_sb


def _front(nc, C, pools, EMB, WR, MBT, t, h_sb):
    """DMA, tok, qkv, attention scores for supertile t."""
    pin, pin2, sb, sb3, sbs, pstat, pout, ps, psb = (
        pools["pin"], pools["pin2"], pools["sb"], pools["sb3"], pools["sbs"],
        pools["pstat"], pools["pout"], pools["ps"], pools["psb"])
    B0 = t * NB
    mm = nc.tensor.matmul
    idf = C["c_id"]
    idb = C["c_idb"]

    # ---- input DMAs ----
    emb_rm = []
    for j in range(2):
        e = pin.tile([128, W], F32, tag=f"emb{j}")
        nc.sync.dma_start(e[:].rearrange("p (s d) -> p s d", s=3),
                          EMB[B0 + j * 128:B0 + (j + 1) * 128])
        emb_rm.append(e)
    w_row = pin2.tile([1, W], F32, tag="w_row")
    nc.sync.dma_start(w_row[:], WR[t:t + 1, :])
    mb_in = pin.tile([3, NB], F32, tag="mb_in")
    nc.sync.dma_start(mb_in[:], MBT[t])

    # ---- h = gelu(w1 x a + b1)  (fm, per chunk) ----
    h_sb = []
    for c in range(2):
        hp = ps.tile([128, W], F32, tag="psA")
        for bk in range(2):
            mm(hp[:, bk * 384:(bk + 1) * 384],
               C["c_w1row"][0:1, c * 128:(c + 1) * 128],
               a_row[0:1, bk * 384:(bk + 1) * 384])
        h = sb.tile([128, W], BF16, tag=f"h{c}")
        nc.scalar.activation(h[:], hp[:], GELU, bias=C["c_spb1"][:, c:c + 1])
        h_sb.append(h)

    if stage == 2:
        return _dbg(nc, OUT, B0, h_sb[0][:, 0:512].bitcast(F32))

    # ---- mask -> rm: wmb cols (j=0: 0:3, j=1: 4:7) ----
    mps = psb.tile([128, W], BF16, tag="psB")
    mpf = mps[:, 0:16].bitcast(F32)  # [128, 8] f32 scratch
    nc.tensor.matmul(mpf[:, 0:3], mb_in[:, 0:128], idf[0:3, 0:3],
                     is_transpose=True)
    nc.tensor.matmul(mpf[:, 4:7], mb_in[:, 128:256], idf[0:3, 0:3],
                     is_transpose=True)
    wmb = sbs.tile([128, 8], F32, tag="wmb")
    nc.vector.tensor_copy(wmb[:], mpf[:, 0:8])

    # ---- tok (fm): emb^T + spw2 @ h  (+species via drain) ----
    tok_sb = []
    for c in range(2):
        tp = ps.tile([128, W], F32, tag="psA")
        blocks = [(s, j) for s in range(3) for j in range(2)]
        for bk in range(2):
            first = True
            for (s, j) in blocks[bk * 3:(bk + 1) * 3]:
                col = s * 256 + j * 128
                mm(tp[:, col:col + 128],
                   emb_rm[j][:, s * 256 + c * 128:s * 256 + c * 128 + 128],
                   idf, is_transpose=True, start=first, stop=False,
                   skip_group_check=True)
                first = False
            for kc in range(2):
                mm(tp[:, bk * 384:(bk + 1) * 384],
                   C["c_spw2t"][:, kc * 256 + c * 128:kc * 256 + c * 128 + 128],
                   h_sb[kc][:, bk * 384:(bk + 1) * 384],
                   start=False, stop=(kc == 1), skip_group_check=True)
        tok = sb3.tile([128, W], BF16, tag=f"tok{c}")
        for s in range(3):
            nc.vector.tensor_scalar(
                out=tok[:, s * 256:(s + 1) * 256],
                in0=tp[:, s * 256:(s + 1) * 256],
                scalar1=C["c_spec"][:, c * 3 + s:c * 3 + s + 1],
                scalar2=None, op0=OP.add)
        tok_sb.append(tok)

    if stage == 1:
        return _dbg(nc, OUT, B0, tok_sb[0][:, 0:512].bitcast(F32))

    # ---- qkv (rm per (j,s)): [q 0:256 | k 256:512 | vk 512:516 | v 516:772] ----
    qkv_sb = []
    for j in range(2):
        q = sb.tile([128, 3 * 772], BF16, tag=f"qkv{j}")
        qkv_sb.append(q)
        for s in range(3):
            qp = ps.tile([128, W], F32, tag="psA")  # 772 <= W pads ok
            qpv = qp[:, 0:772]
            for (lo, hi) in ((0, 512), (512, 772)):
                for kc in range(2):
                    mm(qpv[:, lo:hi],
                       tok_sb[kc][:, s * 256 + j * 128:s * 256 + j * 128 + 128],
                       C["c_qkvw"][:, kc * 772 + lo:kc * 772 + hi],
                       start=(kc == 0), stop=(kc == 1))
            dst = q[:, s * 772:(s + 1) * 772]
            nc.scalar.activation(dst, qpv, AF.Copy)

    if stage == 3:
        return _dbg(nc, OUT, B0, qkv_sb[0][:, 0:512].bitcast(F32))

    # ---- attention scores + softmax (rm, j-merged) ----
    attm = sbs.tile([128, 72], BF16, tag="attm")
    for j in range(2):
        qv = qkv_sb[j][:].rearrange("p (s f) -> p s f", s=3)
        prod = sb.tile([128, 2304], BF16, tag="prod")
        nc.vector.tensor_tensor(
            out=prod[:].rearrange("p (q k f) -> p q k f", q=3, k=3),
            in0=qv[:, :, None, 0:256].broadcast_to((128, 3, 3, 256)),
            in1=qv[:, None, :, 256:512].broadcast_to((128, 3, 3, 256)),
            op=OP.mult)
        with nc.allow_low_precision("bf16 attention scores"):
            nc.vector.tensor_reduce(
                out=attm[:, j * 36:(j + 1) * 36],
                in_=prod[:].rearrange("p (g e) -> p g e", g=36),
                axis=AX.X, op=OP.add)
        av = attm[:, j * 36:(j + 1) * 36].rearrange(
            "p (q k h) -> p q k h", q=3, k=3)
        nc.vector.tensor_tensor(
            out=av, in0=av,
            in1=wmb[:, j * 4:j * 4 + 3][:, None, :, None]
            .broadcast_to((128, 3, 3, 4)), op=OP.add)
        nc.vector.tensor_tensor(
            out=av, in0=av,
            in1=qv[:, None, :, 512:516].broadcast_to((128, 3, 3, 4)),
            op=OP.add)
    # softmax over k, both j at once: views (p, g=j*q, k, h)
    amv = attm[:].rearrange("p (g k h) -> p g k h", g=6, k=3)
    mx = sbs.tile([128, 24], F32, tag="mx")
    mxv = mx[:].rearrange("p (g h) -> p g h", g=6)
    nc.vector.tensor_tensor(out=mxv, in0=amv[:, :, 0], in1=amv[:, :, 1], op=OP.max)
    nc.vector.tensor_tensor(out=mxv, in0=mxv, in1=amv[:, :, 2], op=OP.max)
    es = sbs.tile([128, 72], F32, tag="es")
    esv = es[:].rearrange("p (g k h) -> p g k h", g=6, k=3)
    nc.vector.tensor_tensor(
        out=esv, in0=amv, in1=mxv[:, :, None, :].broadcast_to((128, 6, 3, 4)),
        op=OP.subtract)
    nc.scalar.activation(es[:], es[:], AF.Exp)
    den = sbs.tile([128, 24], F32, tag="den")
    dv = den[:].rearrange("p (g h) -> p g h", g=6)
    nc.vector.tensor_tensor(out=dv, in0=esv[:, :, 0], in1=esv[:, :, 1], op=OP.add)
    nc.vector.tensor_tensor(out=dv, in0=dv, in1=esv[:, :, 2], op=OP.add)
    rden = sbs.tile([128, 24], F32, tag="rden")
    nc.vector.reciprocal(rden[:], den[:])
    p_sb = sbs.tile([128, 72], BF16, tag="p_sb")
    pv = p_sb[:].rearrange("p (g k h) -> p g k h", g=6, k=3)
    nc.vector.tensor_tensor(
        out=pv, in0=esv,
        in1=rden[:].rearrange("p (g h) -> p g h", g=6)[:, :, None, :]
        .broadcast_to((128, 6, 3, 4)), op=OP.mult)

    if stage == 4:
        return _dbg(nc, OUT, B0, es[:, 0:64])

    # ---- mix: o_q = sum_k p_k * v_k  (rm per j) ----
    o_rm = []
    for j in range(2):
        qv = qkv_sb[j][:].rearrange("p (s f) -> p s f", s=3)
        pjv = p_sb[:, j * 36:(j + 1) * 36].rearrange(
            "p (q k h) -> p q k h", q=3, k=3)
        o = sb.tile([128, W], BF16, tag=f"o{j}")
        tmp = sb.tile([128, W], BF16, tag=f"mixt{j}")
        for k in range(3):
            dst = o if k == 0 else tmp
            nc.vector.tensor_tensor(
                out=dst[:].rearrange("p (q h e) -> p q h e", q=3, h=4),
                in0=qv[:, k:k + 1, 516:772].broadcast_to((128, 3, 256))
                .rearrange("p q (h e) -> p q h e", h=4),
                in1=pjv[:, :, k, :][:, :, :, None].broadcast_to((128, 3, 4, 64)),
                op=OP.mult)
            if k > 0:
                nc.vector.tensor_tensor(out=o[:], in0=o[:], in1=tmp[:], op=OP.add)
        o_rm.append(o)

    if stage == 5:
        return _dbg(nc, OUT, B0, o_rm[0][:, 0:512].bitcast(F32))

    # ---- o -> fm ----
    ofm = []
    for c in range(2):
        op_ = psb.tile([128, W], BF16, tag="psB")
        n = 0
        for qq in range(3):
            for j in range(2):
                mm(op_[:, qq * 256 + j * 128:qq * 256 + j * 128 + 128],
                   o_rm[j][:, qq * 256 + c * 128:qq * 256 + c * 128 + 128],
                   idb, is_transpose=True, start=(n == 0), stop=(n == 5),
                   skip_group_check=True)
                n += 1
        of = sb.tile([128, W], BF16, tag=f"ofm{c}")
        nc.vector.tensor_copy(of[:], op_[:])
        ofm.append(of)

    # ---- x1 = o0 @ Wo^T + attC + tok;  + squares ----
    x1_sb, xsq = [], []
    for c in range(2):
        xp = ps.tile([128, W], F32, tag="psA")
        for bk in range(2):
            for kc in range(2):
                mm(xp[:, bk * 384:(bk + 1) * 384],
                   C["c_outwt"][:, kc * 256 + c * 128:kc * 256 + c * 128 + 128],
                   ofm[kc][:, bk * 384:(bk + 1) * 384],
                   start=(kc == 0), stop=(kc == 1))
        x1 = sb.tile([128, W], BF16, tag=f"x1_{c}")
        for bk in range(2):
            nc.vector.scalar_tensor_tensor(
                out=x1[:, bk * 384:(bk + 1) * 384],
                in0=xp[:, bk * 384:(bk + 1) * 384],
                scalar=C["c_attC"][:, c:c + 1],
                in1=tok_sb[c][:, bk * 384:(bk + 1) * 384],
                op0=OP.add, op1=OP.add)
        sq = sb.tile([128, W], BF16, tag=f"sq{c}")
        nc.vector.tensor_tensor(out=sq[:], in0=x1[:], in1=x1[:], op=OP.mult)
        x1_sb.append(x1)
        xsq.append(sq)

    if stage == 6:
        return _dbg(nc, OUT, B0, x1_sb[0][:, 0:512].bitcast(F32))

    # ---- LN1 (fm) ----
    x1na = _ln_apply(nc, C, sb, sb3, pstat, ps, mm, x1_sb, xsq, "1")

    if stage == 7:
        return _dbg(nc, OUT, B0, x1na[0][:, 0:512].bitcast(F32))

    # ---- FFN1: f1 = gelu(x1na @ W1^T + b1') ----
    f1 = sb.tile([128, 4 * W], BF16, tag="f1")
    for oc in range(4):
        fp = ps.tile([128, W], F32, tag="psA")
        for bk in range(2):
            for kc in range(2):
                mm(fp[:, bk * 384:(bk + 1) * 384],
                   C["c_ffn1t"][:, kc * 512 + oc * 128:kc * 512 + oc * 128 + 128],
                   x1na[kc][:, bk * 384:(bk + 1) * 384],
                   start=(kc == 0), stop=(kc == 1))
        nc.scalar.activation(f1[:, oc * W:(oc + 1) * W], fp[:], GELU,
                             bias=C["c_ffn1b"][:, oc:oc + 1])

    if stage == 8:
        return _dbg(nc, OUT, B0, f1[:, 0:512].bitcast(F32))

    st["f1"] = f1
    st["x1na"] = x1na


def _back_b(nc, C, pools, OUT, t, st):
    """FFN2, LN2+pool, output for supertile t."""
    pin, pin2, sb, sb3, sbs, pstat, pout, ps, psb = (
        pools["pin"], pools["pin2"], pools["sb"], pools["sb3"], pools["sbs"],
        pools["pstat"], pools["pout"], pools["ps"], pools["psb"])
    B0 = t * NB
    mm = nc.tensor.matmul
    idf = C["c_id"]
    idb = C["c_idb"]
    w_row = st["w_row"]
    f1, x1na = st["f1"], st["x1na"]

    # ---- FFN2 + resid: x2 = (f1 @ W2^T + b2 + ln1b) + x1na ----
    x2_sb, xsq2 = [], []
    for c in range(2):
        xp = ps.tile([128, W], F32, tag="psA")
        for bk in range(2):
            for kc in range(4):
                mm(xp[:, bk * 384:(bk + 1) * 384],
                   C["c_ffn2t"][:, kc * 256 + c * 128:kc * 256 + c * 128 + 128],
                   f1[:, kc * W + bk * 384:kc * W + (bk + 1) * 384],
                   start=(kc == 0), stop=(kc == 3))
        x2 = sb.tile([128, W], BF16, tag=f"x2_{c}")
        for bk in range(2):
            nc.vector.scalar_tensor_tensor(
                out=x2[:, bk * 384:(bk + 1) * 384],
                in0=xp[:, bk * 384:(bk + 1) * 384],
                scalar=C["c_x2b"][:, c:c + 1],
                in1=x1na[c][:, bk * 384:(bk + 1) * 384],
                op0=OP.add, op1=OP.add)
        sq = sb.tile([128, W], BF16, tag=f"sq{c}")
        nc.vector.tensor_tensor(out=sq[:], in0=x2[:], in1=x2[:], op=OP.mult)
        x2_sb.append(x2)
        xsq2.append(sq)

    if stage == 9:
        return _dbg(nc, OUT, B0, x2_sb[0][:, 0:512].bitcast(F32))

    # ---- LN2 fused with masked-mean pool ----
    sums = _ln_sums(nc, C, ps, mm, x2_sb, xsq2)
    rstd, sta = _ln_stats(nc, C, pstat, sums, "2")
    c_r = pstat.tile([1, W], F32R, tag="stc_2")
    nc.vector.tensor_tensor(out=c_r[:], in0=w_row[:], in1=rstd[:], op=OP.mult)
    t2 = rstd  # rstd dead after c_r; reuse as t2 scratch
    nc.vector.tensor_tensor(out=t2[:], in0=c_r[:], in1=sums[0:1, :], op=OP.mult)
    d_r = pstat.tile([1, 256], F32R, tag="d_r")
    nc.vector.tensor_tensor(out=d_r[:], in0=t2[0:1, 0:256],
                            in1=t2[0:1, 256:512], op=OP.add)
    nc.vector.tensor_tensor(out=d_r[:], in0=d_r[:], in1=t2[0:1, 512:768],
                            op=OP.add)
    c2p = ps.tile([128, W], F32, tag="psA")
    for bk in range(2):
        mm(c2p[:, bk * 384:(bk + 1) * 384], C["c_ones1"],
           c_r[0:1, bk * 384:(bk + 1) * 384])
    dgp = []
    for c in range(2):
        dp = psb.tile([128, W], BF16, tag="psB")
        dpf = dp[:, 0:512].bitcast(F32)
        mm(dpf[:, 0:256], C["c_ng2row"][0:1, c * 128:(c + 1) * 128], d_r[:],
           start=True, stop=False, skip_group_check=True)
        mm(dpf[:, 0:256], C["c_b2row"][0:1, c * 128:(c + 1) * 128],
           C["c_onesrow"][0:1, 0:256], start=False, stop=True,
           skip_group_check=True)
        dgp.append(dpf)
    outfm = []
    for c in range(2):
        pt = sb.tile([128, W], BF16, tag=f"sq{c}")
        nc.vector.tensor_tensor(out=pt[:], in0=x2_sb[c][:], in1=c2p[:],
                                op=OP.mult)
        a1 = sb.tile([128, 256], BF16, tag=f"pa{c}")
        nc.vector.tensor_tensor(out=a1[:], in0=pt[:, 0:256],
                                in1=pt[:, 256:512], op=OP.add)
        nc.vector.tensor_tensor(out=a1[:], in0=a1[:], in1=pt[:, 512:768],
                                op=OP.add)
        of = pout.tile([128, 256], F32, tag=f"outfm{c}")
        nc.vector.scalar_tensor_tensor(
            out=of[:], in0=a1[:], scalar=C["c_g2pp"][:, c:c + 1],
            in1=dgp[c][:, 0:256], op0=OP.mult, op1=OP.add)
        outfm.append(of)
    for j in range(2):
        orp = psb.tile([128, W], BF16, tag="psB")
        orf = orp[:, 0:512].bitcast(F32)
        for c in range(2):
            mm(orf[:, c * 128:(c + 1) * 128],
               outfm[c][:, j * 128:(j + 1) * 128], idf,
               is_transpose=True, start=(c == 0), stop=(c == 1),
               skip_group_check=True)
        ot = pout.tile([128, 256], F32, tag=f"out{j}")
        nc.scalar.activation(ot[:], orf[:], AF.Copy)
        nc.sync.dma_start(OUT[B0 + j * 128:B0 + (j + 1) * 128, :], ot[:])


_CACHE: dict = {}


def _get_nc(bl: int) -> bass.Bass:
    if bl not in _CACHE:
        _CACHE[bl] = build(bl)
    return _CACHE[bl]


LAST_RESULT = None


def kernel(**inputs) -> np.ndarray:
    global LAST_RESULT
    consts = host_consts(inputs)
    nc = _get_nc(B // NCORES)
    in_maps = [host_inputs(i, inputs, consts) for i in range(NCORES)]
    res = run_bass_kernel_spmd(nc, in_maps, core_ids=list(range(NCORES)))
    LAST_RESULT = res
    return np.concatenate([r["out"] for r in res.results], axis=0)


# revision 24
# speedup vs baseline: 1.5527x; 1.0049x over previous
"""Trainium2 Bass kernel for nn_CrossOrganismAttention (v2).

Data-parallel over 8 cores (batch sharded). Per core, rows processed in
supertiles of NB=256 batch rows (= 768 (s,b) columns feature-major, or
2 row-major blocks of 128). Feature-major main stream; attention/softmax
row-major. qkv/out biases folded via softmax invariance (only a 4-col
"vk" weight extension survives); species/residual/bias adds ride on
vector drains; LN stats via PE ones-matmuls; rstd via Exp(-0.5*Ln(var+eps))
so all non-Gelu ACT work stays in one table set; LN2 fused into the
masked-mean pool.
"""

import numpy as np

import concourse.bass as bass
import concourse.mybir as mybir
from concourse.bacc import Bacc
from concourse.tile import TileContext
from concourse.bass_utils import run_bass_kernel_spmd

B, S, D, H, DH = 65536, 3, 256, 4, 64
NCORES = 8
NB = 256  # batch rows per supertile (2 row-major blocks)
W = 768  # feature-major column count per chunk: (s, b) = 3*256
EPS = 1e-5
F32 = mybir.dt.float32
F32R = mybir.dt.float32r
BF16 = mybir.dt.bfloat16
AF = mybir.ActivationFunctionType
OP = mybir.AluOpType
AX = mybir.AxisListType
GELU = AF.Gelu


def _chunk_lhst(w_t: np.ndarray) -> np.ndarray:
    """(K, M) lhsT -> (128, K//128 * M) with chunk c at cols [c*M:(c+1)*M]."""
    k, m = w_t.shape
    assert k % 128 == 0
    return np.ascontiguousarray(
        w_t.reshape(k // 128, 128, m).transpose(1, 0, 2).reshape(128, -1)
    )


def _per_part(v: np.ndarray) -> np.ndarray:
    """(n*128,) vector -> (128, n) with chunk c in col c."""
    n = v.shape[0] // 128
    return np.ascontiguousarray(v.reshape(n, 128).T)


# Packed constant layouts: (key, cols).
_PACK_BF16 = [("c_qkvw", 2 * 772), ("c_spw2t", 512), ("c_outwt", 512),
              ("c_ffn1t", 1024), ("c_ffn2t", 1024), ("c_idb", 128),
              ("c_recipD", 1)]
_PACK_F32 = [("c_id", 128), ("c_spec", 6), ("c_spb1", 2), ("c_ffn1b", 4),
             ("c_x2b", 2), ("c_attC", 2), ("c_g1pp", 2), ("c_g2pp", 2)]
_PACK_SMALL = [("c_w1row", 256), ("c_ones1", 128), ("c_onesrow", 768),
               ("c_g1row", 256), ("c_ng2row", 256), ("c_b2row", 256),
               ("c_eps", 1)]


def host_consts(p: dict) -> dict:
    f = lambda x: np.asarray(x, np.float32)
    sq = 1.0 / np.sqrt(DH)
    in_w, in_b = f(p["in_w"]), f(p["in_b"])
    out_w, out_b = f(p["out_w"]), f(p["out_b"])
    sp_w1, sp_b1 = f(p["sp_w1"]), f(p["sp_b1"])
    sp_w2, sp_b2 = f(p["sp_w2"]), f(p["sp_b2"])
    ffn_w1, ffn_b1 = f(p["ffn_w1"]), f(p["ffn_b1"])
    ffn_w2, ffn_b2 = f(p["ffn_w2"]), f(p["ffn_b2"])
    ln1_g, ln1_b = f(p["ln1_g"]), f(p["ln1_b"])
    ln2_g, ln2_b = f(p["ln2_g"]), f(p["ln2_b"])
    species = f(p["species_emb"])

    wq = in_w[0:256] * sq
    bq = in_b[0:256] * sq
    wk = in_w[256:512]
    wv = in_w[512:768]
    bv = in_b[512:768]
    # vk[d, h] = wk_h^T @ bq_h  (the only qkv-bias term softmax keeps)
    wvk = np.zeros((256, 4), np.float32)
    for h in range(H):
        wvk[:, h] = wk[h * DH:(h + 1) * DH].T @ bq[h * DH:(h + 1) * DH]
    # qkv weights (256, 772): [q 0:256 | k 256:512 | vk 512:516 | v 516:772]
    wqkv = np.concatenate([wq.T, wk.T, wvk, wv.T], axis=1)
    attC = out_w @ bv + out_b  # x1 = o0 @ Wo^T + attC + tok

    consts = {
        "c_qkvw": _chunk_lhst(wqkv),                      # (128, 2*772)
        "c_spw2t": _chunk_lhst(sp_w2.T),                  # (128, 2*256)
        "c_outwt": _chunk_lhst(out_w.T),                  # (128, 2*256)
        "c_ffn1t": _chunk_lhst(ffn_w1.T),                 # (128, 2*512)
        "c_ffn2t": _chunk_lhst(ffn_w2.T),                 # (128, 4*256)
        "c_idb": np.eye(128, dtype=np.float32),
        "c_recipD": np.full((128, 1), 1.0 / 256.0, np.float32),
        "c_id": np.eye(128, dtype=np.float32),
        "c_spec": np.ascontiguousarray(
            (species + sp_b2[None, :]).T.reshape(2, 128, 3)
            .transpose(1, 0, 2).reshape(128, 6)),          # (128, (c,s))
        "c_spb1": _per_part(sp_b1),
        "c_ffn1b": _per_part(ffn_b1 + ffn_w1 @ ln1_b),    # ln1_b folded
        "c_x2b": _per_part(ffn_b2 + ln1_b),
        "c_attC": _per_part(attC),
        "c_g1pp": _per_part(ln1_g),
        "c_g2pp": _per_part(ln2_g),
        "c_w1row": np.ascontiguousarray(sp_w1[:, 0][None, :]),   # (1, 256)
        "c_ones1": np.ones((1, 128), np.float32),
        "c_onesrow": np.ones((1, 768), np.float32),
        "c_g1row": np.ascontiguousarray(ln1_g[None, :]),  # (1, 256)
        "c_ng2row": np.ascontiguousarray(-ln2_g[None, :]),
        "c_b2row": np.ascontiguousarray(ln2_b[None, :]),
        "c_eps": np.full((1, 1), EPS, np.float32),
    }
    import ml_dtypes
    for k, _ in _PACK_BF16:
        consts[k] = consts[k].astype(ml_dtypes.bfloat16).view(np.uint16)

    def pack(layout):
        return np.ascontiguousarray(
            np.concatenate([consts[k] for k, _ in layout], axis=1))

    return {
        "c_pbf16": pack(_PACK_BF16),
        "c_pf32": pack(_PACK_F32).astype(np.float32),
        "c_small": pack(_PACK_SMALL).astype(np.float32),
    }


def host_inputs(core: int, inputs: dict, consts: dict) -> dict:
    bl = B // NCORES
    b0 = core * bl
    nst = bl // NB
    emb = np.ascontiguousarray(
        np.asarray(inputs["organism_embeddings"], np.float32)[b0:b0 + bl])
    a = np.asarray(inputs["anomaly_scores"], np.float32)[b0:b0 + bl]
    a_t = np.ascontiguousarray(
        a.reshape(nst, NB, S).transpose(0, 2, 1).reshape(nst, S * NB))
    mask = np.asarray(inputs["organism_mask"], bool)[b0:b0 + bl]
    valid = (~mask).astype(np.float32)
    w = valid / valid.sum(axis=1, keepdims=True)
    wrow = np.ascontiguousarray(
        w.reshape(nst, NB, S).transpose(0, 2, 1).reshape(nst, S * NB))
    mb = (mask.astype(np.float32) * -1e9)
    mbt = np.ascontiguousarray(
        mb.reshape(nst, NB, S).transpose(0, 2, 1))  # (nst, 3, 256)
    m = {"emb": emb, "a": a_t, "wr": wrow, "mbt": mbt}
    m.update(consts)
    return m


def build(bl: int, stage=None) -> bass.Bass:
    nst = bl // NB
    nc = Bacc()

    EMB = nc.declare_dram_parameter("emb", [bl, S, D], F32, isOutput=False)
    A = nc.declare_dram_parameter("a", [nst, S * NB], F32R, isOutput=False)
    WR = nc.declare_dram_parameter("wr", [nst, S * NB], F32, isOutput=False)
    MBT = nc.declare_dram_parameter("mbt", [nst, S, NB], F32, isOutput=False)
    OUT = nc.declare_dram_parameter("out", [bl, D], F32, isOutput=True)

    packs = {
        "c_pbf16": (_PACK_BF16, BF16, 128),
        "c_pf32": (_PACK_F32, F32, 128),
        "c_small": (_PACK_SMALL, F32R, 1),
    }
    cparams = {}
    for pk, (layout, dt, rows) in packs.items():
        wdt = sum(n for _, n in layout)
        pdt = mybir.dt.uint16 if dt == BF16 else dt
        cparams[pk] = nc.declare_dram_parameter(pk, [rows, wdt], pdt, isOutput=False)

    with TileContext(nc) as tc:
        with (
            tc.tile_pool(name="consts", bufs=1) as cpool,
            tc.tile_pool(name="pin", bufs=3) as pin,
            tc.tile_pool(name="pin2", bufs=2) as pin2,
            tc.tile_pool(name="sb", bufs=2) as sb,
            tc.tile_pool(name="sb3", bufs=3) as sb3,
            tc.tile_pool(name="sbs", bufs=3) as sbs,
            tc.tile_pool(name="pstat", bufs=2) as pstat,
            tc.tile_pool(name="pout", bufs=3) as pout,
            tc.tile_pool(name="ps", bufs=3, space="PSUM") as ps,
            tc.tile_pool(name="psb", bufs=2, space="PSUM") as psb,
        ):
            C = {}
            for pk, (layout, dt, rows) in packs.items():
                wdt = sum(n for _, n in layout)
                ct = cpool.tile([rows, wdt], dt, tag=pk)
                src = cparams[pk][:, :]
                if dt == BF16:
                    src = src.bitcast(BF16)
                nc.sync.dma_start(ct[:], src)
                off = 0
                for k, n in layout:
                    C[k] = ct[:, off:off + n]
                    off += n
            _warmup(nc, C, ps, psb, sbs)
            pools = dict(pin=pin, pin2=pin2, sb=sb, sb3=sb3, sbs=sbs,
                         pstat=pstat, pout=pout, ps=ps, psb=psb)
            h_next = _h_phase(nc, C, pools, A, 0)
            st_prev = None
            for t in range(nst + 1):
                st = None
                if t < nst:
                    st = _front(nc, C, pools, EMB, WR, MBT, t, h_next)
                if st_prev is not None:
                    _back_a(nc, C, pools, t - 1, st_prev)
                if t + 1 < nst:
                    h_next = _h_phase(nc, C, pools, A, t + 1)
                if st_prev is not None:
                    _back_b(nc, C, pools, OUT, t - 1, st_prev)
                st_prev = st
    nc.compile()
    return nc


def _warmup(nc, C, ps, psb, sbs):
    """Touch each const pack once on PE/DVE/ACT so later ops have at most
    one fresh semaphore wait."""
    wps = ps.tile([128, W], F32, tag="psA")
    nc.tensor.transpose(wps[:, 0:128], C["c_id"], C["c_id"])
    nc.tensor.matmul(wps[:, 0:128], C["c_qkvw"][:, 0:128], C["c_idb"])
    nc.tensor.matmul(wps[:, 0:384], C["c_ones1"], C["c_onesrow"][0:1, 0:384])
    wpb = psb.tile([128, W], BF16, tag="psB")
    nc.tensor.transpose(wpb[:, 0:128], C["c_idb"], C["c_idb"])
    wsc = sbs.tile([128, 8], F32, tag="wsc")
    nc.vector.tensor_copy(wsc[:, 0:1], C["c_id"][:, 0:1])
    nc.vector.tensor_copy(wsc[:, 1:2], C["c_idb"][:, 0:1])
    nc.vector.tensor_copy(wsc[0:1, 2:3], C["c_w1row"][0:1, 0:1])
    nc.scalar.activation(wsc[:, 4:5], C["c_id"][:, 1:2], AF.Copy)
    nc.scalar.activation(wsc[:, 5:6], C["c_idb"][:, 1:2], AF.Copy)
    nc.scalar.activation(wsc[0:1, 6:7], C["c_w1row"][0:1, 1:2], AF.Copy)


def _dbg(nc, OUT, B0, ap):
    nc.sync.dma_start(OUT[B0:B0 + 128, :], ap)


def _ln_sums(nc, C, ps, mm, x_sb, xsq):
    """psum [2, W]: row0 = mean(x), row1 = E[x^2]  (1/256 folded in lhsT)."""
    sums = ps.tile([33, W], F32, tag="psA")  # row 0 = mean, row 32 = E[x^2]
    for bk in range(2):
        sl = slice(bk * 384, (bk + 1) * 384)
        for c in range(2):
            mm(sums[0:1, sl], C["c_recipD"], x_sb[c][:, sl],
               start=(c == 0), stop=(c == 1))
        for c in range(2):
            mm(sums[32:33, sl], C["c_recipD"], xsq[c][:, sl],
               start=(c == 0), stop=(c == 1), skip_group_check=True)
    return sums


def _ln_stats(nc, C, pstat, sums, sfx):
    """rstd from sums via in-place chain; returns (rstd, sta_scratch)."""
    sta = pstat.tile([1, W], F32, tag="sta_" + sfx)
    nc.scalar.activation(sta[:], sums[0:1, :], AF.Square)
    nc.vector.tensor_tensor(out=sta[:], in0=sums[32:33, :], in1=sta[:],
                            op=OP.subtract)
    nc.scalar.activation(sta[:], sta[:], AF.Ln, bias=C["c_eps"][0:1, 0:1])
    rstd = pstat.tile([1, W], F32R, tag="stb_" + sfx)
    nc.scalar.activation(rstd[:], sta[:], AF.Exp, scale=-0.5)
    return rstd, sta


def _ln_apply(nc, C, sb, sb3, pstat, ps, mm, x_sb, xsq, sfx):
    """x1na_c = g1_c * ((x_c - m) * rstd) + 0  (ln1_b folded downstream)."""
    sums = _ln_sums(nc, C, ps, mm, x_sb, xsq)
    rstd, sta = _ln_stats(nc, C, pstat, sums, sfx)
    negmr = pstat.tile([1, W], F32R, tag="stc_" + sfx)
    nc.vector.scalar_tensor_tensor(out=negmr[:], in0=sums[0:1, :], scalar=-1.0,
                                   in1=rstd[:], op0=OP.mult, op1=OP.mult)
    rp = ps.tile([128, W], F32, tag="psA")
    for bk in range(2):
        sl = slice(bk * 384, (bk + 1) * 384)
        mm(rp[:, sl], C["c_ones1"], rstd[0:1, sl])
    out = []
    for c in range(2):
        wp = ps.tile([128, W], F32, tag="psA")
        for bk in range(2):
            sl = slice(bk * 384, (bk + 1) * 384)
            mm(wp[:, sl], C["c_g1row"][0:1, c * 128:(c + 1) * 128],
               negmr[0:1, sl])
        tt = sb.tile([128, W], BF16, tag=f"lnt{sfx}{c}")
        nc.vector.tensor_tensor(out=tt[:], in0=x_sb[c][:], in1=rp[:], op=OP.mult)
        xa = sb.tile([128, W], BF16, tag=f"xna{sfx}{c}")
        nc.vector.scalar_tensor_tensor(
            out=xa[:], in0=tt[:], scalar=C["c_g1pp"][:, c:c + 1],
            in1=wp[:], op0=OP.mult, op1=OP.add)
        out.append(xa)
    return out


def _h_phase(nc, C, pools, A, t):
    """score-MLP for supertile t: h = gelu(w1 x a + b1), fm per chunk."""
    pin2, sb, ps = pools["pin2"], pools["sb"], pools["ps"]
    mm = nc.tensor.matmul
    a_row = pin2.tile([1, W], F32R, tag="a_row")
    nc.sync.dma_start(a_row[:], A[t:t + 1, :])
    h_sb = []
    for c in range(2):
        hp = ps.tile([128, W], F32, tag="psA")
        for (lo, hi) in PIECES:
            mm(hp[:, lo:hi],
               C["c_w1row"][0:1, c * 128:(c + 1) * 128],
               a_row[0:1, lo:hi])
        h = sb.tile([128, W], BF16, tag=f"h{c}")
        nc.scalar.activation(h[:], hp[:], GELU, bias=C["c_spb1"][:, c:c + 1])
        h_sb.append(h)
    return h_sb


def _front(nc, C, pools, EMB, WR, MBT, t, h_sb):
    """DMA, tok, qkv, attention scores for supertile t."""
    pin, pin2, sb, sb3, sbs, pstat, pout, ps, psb = (
        pools["pin"], pools["pin2"], pools["sb"], pools["sb3"], pools["sbs"],
        pools["pstat"], pools["pout"], pools["ps"], pools["psb"])
    B0 = t * NB
    mm = nc.tensor.matmul
    idf = C["c_id"]
    idb = C["c_idb"]

    # ---- input DMAs ----
    emb_rm = []
    for j in range(2):
        e = pin.tile([128, W], F32, tag=f"emb{j}")
        nc.sync.dma_start(e[:].rearrange("p (s d) -> p s d", s=3),
                          EMB[B0 + j * 128:B0 + (j + 1) * 128])
        emb_rm.append(e)
    w_row = pin2.tile([1, W], F32, tag="w_row")
    nc.sync.dma_start(w_row[:], WR[t:t + 1, :])
    mb_in = pin.tile([3, NB], F32, tag="mb_in")
    nc.sync.dma_start(mb_in[:], MBT[t])

    # ---- h = gelu(w1 x a + b1)  (fm, per chunk) ----
    h_sb = []
    for c in range(2):
        hp = ps.tile([128, W], F32, tag="psA")
        for bk in range(2):
            mm(hp[:, bk * 384:(bk + 1) * 384],
               C["c_w1row"][0:1, c * 128:(c + 1) * 128],
               a_row[0:1, bk * 384:(bk + 1) * 384])
        h = sb.tile([128, W], BF16, tag=f"h{c}")
        nc.scalar.activation(h[:], hp[:], GELU, bias=C["c_spb1"][:, c:c + 1])
        h_sb.append(h)

    if stage == 2:
        return _dbg(nc, OUT, B0, h_sb[0][:, 0:512].bitcast(F32))

    # ---- mask -> rm: wmb cols (j=0: 0:3, j=1: 4:7) ----
    mps = psb.tile([128, W], BF16, tag="psB")
    mpf = mps[:, 0:16].bitcast(F32)  # [128, 8] f32 scratch
    nc.tensor.matmul(mpf[:, 0:3], mb_in[:, 0:128], idf[0:3, 0:3],
                     is_transpose=True)
    nc.tensor.matmul(mpf[:, 4:7], mb_in[:, 128:256], idf[0:3, 0:3],
                     is_transpose=True)
    wmb = sbs.tile([128, 8], F32, tag="wmb")
    nc.vector.tensor_copy(wmb[:], mpf[:, 0:8])

    # ---- tok (fm): emb^T + spw2 @ h  (+species via drain) ----
    tok_sb = []
    for c in range(2):
        tp = ps.tile([128, W], F32, tag="psA")
        blocks = [(s, j) for s in range(3) for j in range(2)]
        for bk in range(2):
            first = True
            for (s, j) in blocks[bk * 3:(bk + 1) * 3]:
                col = s * 256 + j * 128
                mm(tp[:, col:col + 128],
                   emb_rm[j][:, s * 256 + c * 128:s * 256 + c * 128 + 128],
                   idf, is_transpose=True, start=first, stop=False,
                   skip_group_check=True)
                first = False
            for kc in range(2):
                mm(tp[:, bk * 384:(bk + 1) * 384],
                   C["c_spw2t"][:, kc * 256 + c * 128:kc * 256 + c * 128 + 128],
                   h_sb[kc][:, bk * 384:(bk + 1) * 384],
                   start=False, stop=(kc == 1), skip_group_check=True)
        tok = sb3.tile([128, W], BF16, tag=f"tok{c}")
        for s in range(3):
            nc.vector.tensor_scalar(
                out=tok[:, s * 256:(s + 1) * 256],
                in0=tp[:, s * 256:(s + 1) * 256],
                scalar1=C["c_spec"][:, c * 3 + s:c * 3 + s + 1],
                scalar2=None, op0=OP.add)
        tok_sb.append(tok)

    if stage == 1:
        return _dbg(nc, OUT, B0, tok_sb[0][:, 0:512].bitcast(F32))

    # ---- qkv (rm per (j,s)): [q 0:256 | k 256:512 | vk 512:516 | v 516:772] ----
    qkv_sb = []
    for j in range(2):
        q = sb.tile([128, 3 * 772], BF16, tag=f"qkv{j}")
        qkv_sb.append(q)
        for s in range(3):
            qp = ps.tile([128, W], F32, tag="psA")  # 772 <= W pads ok
            qpv = qp[:, 0:772]
            for (lo, hi) in ((0, 512), (512, 772)):
                for kc in range(2):
                    mm(qpv[:, lo:hi],
                       tok_sb[kc][:, s * 256 + j * 128:s * 256 + j * 128 + 128],
                       C["c_qkvw"][:, kc * 772 + lo:kc * 772 + hi],
                       start=(kc == 0), stop=(kc == 1))
            dst = q[:, s * 772:(s + 1) * 772]
            nc.scalar.activation(dst, qpv, AF.Copy)

    if stage == 3:
        return _dbg(nc, OUT, B0, qkv_sb[0][:, 0:512].bitcast(F32))

    # ---- attention scores + softmax (rm, j-merged) ----
    attm = sbs.tile([128, 72], BF16, tag="attm")
    for j in range(2):
        qv = qkv_sb[j][:].rearrange("p (s f) -> p s f", s=3)
        prod = sb.tile([128, 2304], BF16, tag="prod")
        nc.vector.tensor_tensor(
            out=prod[:].rearrange("p (q k f) -> p q k f", q=3, k=3),
            in0=qv[:, :, None, 0:256].broadcast_to((128, 3, 3, 256)),
            in1=qv[:, None, :, 256:512].broadcast_to((128, 3, 3, 256)),
            op=OP.mult)
        pf = prod[:].rearrange("p (g x e) -> p g x e", g=36, x=2)
        nc.vector.tensor_tensor(
            out=prod[:, 0:1152].rearrange("p (g e) -> p g e", g=36),
            in0=pf[:, :, 0], in1=pf[:, :, 1], op=OP.add)
        with nc.allow_low_precision("bf16 attention scores"):
            nc.vector.tensor_reduce(
                out=attm[:, j * 36:(j + 1) * 36],
                in_=prod[:, 0:1152].rearrange("p (g e) -> p g e", g=36),
                axis=AX.X, op=OP.add)
        av = attm[:, j * 36:(j + 1) * 36].rearrange(
            "p (q k h) -> p q k h", q=3, k=3)
        nc.vector.tensor_tensor(
            out=av, in0=av,
            in1=wmb[:, j * 4:j * 4 + 3][:, None, :, None]
            .broadcast_to((128, 3, 3, 4)), op=OP.add)
        nc.vector.tensor_tensor(
            out=av, in0=av,
            in1=qv[:, None, :, 512:516].broadcast_to((128, 3, 3, 4)),
            op=OP.add)
    # softmax over k, both j at once: views (p, g=j*q, k, h)
    amv = attm[:].rearrange("p (g k h) -> p g k h", g=6, k=3)
    mx = sbs.tile([128, 24], F32, tag="mx")
    mxv = mx[:].rearrange("p (g h) -> p g h", g=6)
    nc.vector.tensor_tensor(out=mxv, in0=amv[:, :, 0], in1=amv[:, :, 1], op=OP.max)
    nc.vector.tensor_tensor(out=mxv, in0=mxv, in1=amv[:, :, 2], op=OP.max)
    es = sbs.tile([128, 72], F32, tag="es")
    esv = es[:].rearrange("p (g k h) -> p g k h", g=6, k=3)
    nc.vector.tensor_tensor(
        out=esv, in0=amv, in1=mxv[:, :, None, :].broadcast_to((128, 6, 3, 4)),
        op=OP.subtract)
    nc.scalar.activation(es[:], es[:], AF.Exp)
    den = sbs.tile([128, 24], F32, tag="den")
    dv = den[:].rearrange("p (g h) -> p g h", g=6)
    nc.vector.tensor_tensor(out=dv, in0=esv[:, :, 0], in1=esv[:, :, 1], op=OP.add)
    nc.vector.tensor_tensor(out=dv, in0=dv, in1=esv[:, :, 2], op=OP.add)
    rden = sbs.tile([128, 24], F32, tag="rden")
    nc.vector.reciprocal(rden[:], den[:])
    p_sb = sbs.tile([128, 72], BF16, tag="p_sb")
    pv = p_sb[:].rearrange("p (g k h) -> p g k h", g=6, k=3)
    nc.vector.tensor_tensor(
        out=pv, in0=esv,
        in1=rden[:].rearrange("p (g h) -> p g h", g=6)[:, :, None, :]
        .broadcast_to((128, 6, 3, 4)), op=OP.mult)

    if stage == 4:
        return _dbg(nc, OUT, B0, es[:, 0:64])

    # ---- mix: o_q = sum_k p_k * v_k  (rm per j) ----
    o_rm = []
    for j in range(2):
        qv = qkv_sb[j][:].rearrange("p (s f) -> p s f", s=3)
        pjv = p_sb[:, j * 36:(j + 1) * 36].rearrange(
            "p (q k h) -> p q k h", q=3, k=3)
        o = sb.tile([128, W], BF16, tag=f"o{j}")
        tmp = sb.tile([128, W], BF16, tag=f"mixt{j}")
        for k in range(3):
            dst = o if k == 0 else tmp
            nc.vector.tensor_tensor(
                out=dst[:].rearrange("p (q h e) -> p q h e", q=3, h=4),
                in0=qv[:, k:k + 1, 516:772].broadcast_to((128, 3, 256))
                .rearrange("p q (h e) -> p q h e", h=4),
                in1=pjv[:, :, k, :][:, :, :, None].broadcast_to((128, 3, 4, 64)),
                op=OP.mult)
            if k > 0:
                nc.vector.tensor_tensor(out=o[:], in0=o[:], in1=tmp[:], op=OP.add)
        o_rm.append(o)

    if stage == 5:
        return _dbg(nc, OUT, B0, o_rm[0][:, 0:512].bitcast(F32))

    # ---- o -> fm ----
    ofm = []
    for c in range(2):
        op_ = psb.tile([128, W], BF16, tag="psB")
        n = 0
        for qq in range(3):
            for j in range(2):
                mm(op_[:, qq * 256 + j * 128:qq * 256 + j * 128 + 128],
                   o_rm[j][:, qq * 256 + c * 128:qq * 256 + c * 128 + 128],
                   idb, is_transpose=True, start=(n == 0), stop=(n == 5),
                   skip_group_check=True)
                n += 1
        of = sb.tile([128, W], BF16, tag=f"ofm{c}")
        nc.vector.tensor_copy(of[:], op_[:])
        ofm.append(of)

    # ---- x1 = o0 @ Wo^T + attC + tok;  + squares ----
    x1_sb, xsq = [], []
    for c in range(2):
        xp = ps.tile([128, W], F32, tag="psA")
        for bk in range(2):
            for kc in range(2):
                mm(xp[:, bk * 384:(bk + 1) * 384],
                   C["c_outwt"][:, kc * 256 + c * 128:kc * 256 + c * 128 + 128],
                   ofm[kc][:, bk * 384:(bk + 1) * 384],
                   start=(kc == 0), stop=(kc == 1))
        x1 = sb.tile([128, W], BF16, tag=f"x1_{c}")
        for bk in range(2):
            nc.vector.scalar_tensor_tensor(
                out=x1[:, bk * 384:(bk + 1) * 384],
                in0=xp[:, bk * 384:(bk + 1) * 384],
                scalar=C["c_attC"][:, c:c + 1],
                in1=tok_sb[c][:, bk * 384:(bk + 1) * 384],
                op0=OP.add, op1=OP.add)
        sq = sb.tile([128, W], BF16, tag=f"sq{c}")
        nc.vector.tensor_tensor(out=sq[:], in0=x1[:], in1=x1[:], op=OP.mult)
        x1_sb.append(x1)
        xsq.append(sq)

    if stage == 6:
        return _dbg(nc, OUT, B0, x1_sb[0][:, 0:512].bitcast(F32))

    # ---- LN1 (fm) ----
    x1na = _ln_apply(nc, C, sb, sb3, pstat, ps, mm, x1_sb, xsq, "1")

    if stage == 7:
        return _dbg(nc, OUT, B0, x1na[0][:, 0:512].bitcast(F32))

    # ---- FFN1: f1 = gelu(x1na @ W1^T + b1') ----
    f1 = sb.tile([128, 4 * W], BF16, tag="f1")
    for oc in range(4):
        fp = ps.tile([128, W], F32, tag="psA")
        for bk in range(2):
            for kc in range(2):
                mm(fp[:, bk * 384:(bk + 1) * 384],
                   C["c_ffn1t"][:, kc * 512 + oc * 128:kc * 512 + oc * 128 + 128],
                   x1na[kc][:, bk * 384:(bk + 1) * 384],
                   start=(kc == 0), stop=(kc == 1))
        nc.scalar.activation(f1[:, oc * W:(oc + 1) * W], fp[:], GELU,
                             bias=C["c_ffn1b"][:, oc:oc + 1])

    if stage == 8:
        return _dbg(nc, OUT, B0, f1[:, 0:512].bitcast(F32))

    st["f1"] = f1
    st["x1na"] = x1na


def _back_b(nc, C, pools, OUT, t, st):
    """FFN2, LN2+pool, output for supertile t."""
    pin, pin2, sb, sb3, sbs, pstat, pout, ps, psb = (
        pools["pin"], pools["pin2"], pools["sb"], pools["sb3"], pools["sbs"],
        pools["pstat"], pools["pout"], pools["ps"], pools["psb"])
    B0 = t * NB
    mm = nc.tensor.matmul
    idf = C["c_id"]
    idb = C["c_idb"]
    w_row = st["w_row"]
    f1, x1na = st["f1"], st["x1na"]

    # ---- FFN2 + resid: x2 = (f1 @ W2^T + b2 + ln1b) + x1na ----
    x2_sb, xsq2 = [], []
    for c in range(2):
        xp = ps.tile([128, W], F32, tag="psA")
        for bk in range(2):
            for kc in range(4):
                mm(xp[:, bk * 384:(bk + 1) * 384],
                   C["c_ffn2t"][:, kc * 256 + c * 128:kc * 256 + c * 128 + 128],
                   f1[:, kc * W + bk * 384:kc * W + (bk + 1) * 384],
                   start=(kc == 0), stop=(kc == 3))
        x2 = sb.tile([128, W], BF16, tag=f"x2_{c}")
        for bk in range(2):
            nc.vector.scalar_tensor_tensor(
                out=x2[:, bk * 384:(bk + 1) * 384],
                in0=xp[:, bk * 384:(bk + 1) * 384],
                scalar=C["c_x2b"][:, c:c + 1],
                in1=x1na[c][:, bk * 384:(bk + 1) * 384],
                op0=OP.add, op1=OP.add)
        sq = sb.tile([128, W], BF16, tag=f"sq{c}")
        nc.vector.tensor_tensor(out=sq[:], in0=x2[:], in1=x2[:], op=OP.mult)
        x2_sb.append(x2)
        xsq2.append(sq)

    if stage == 9:
        return _dbg(nc, OUT, B0, x2_sb[0][:, 0:512].bitcast(F32))

    # ---- LN2 fused with masked-mean pool ----
    sums = _ln_sums(nc, C, ps, mm, x2_sb, xsq2)
    rstd, sta = _ln_stats(nc, C, pstat, sums, "2")
    c_r = pstat.tile([1, W], F32R, tag="stc_2")
    nc.vector.tensor_tensor(out=c_r[:], in0=w_row[:], in1=rstd[:], op=OP.mult)
    t2 = rstd  # rstd dead after c_r; reuse as t2 scratch
    nc.vector.tensor_tensor(out=t2[:], in0=c_r[:], in1=sums[0:1, :], op=OP.mult)
    d_r = pstat.tile([1, 256], F32R, tag="d_r")
    nc.vector.tensor_tensor(out=d_r[:], in0=t2[0:1, 0:256],
                            in1=t2[0:1, 256:512], op=OP.add)
    nc.vector.tensor_tensor(out=d_r[:], in0=d_r[:], in1=t2[0:1, 512:768],
                            op=OP.add)
    c2p = ps.tile([128, W], F32, tag="psA")
    for bk in range(2):
        mm(c2p[:, bk * 384:(bk + 1) * 384], C["c_ones1"],
           c_r[0:1, bk * 384:(bk + 1) * 384])
    dgp = []
    for c in range(2):
        dp = psb.tile([128, W], BF16, tag="psB")
        dpf = dp[:, 0:512].bitcast(F32)
        mm(dpf[:, 0:256], C["c_ng2row"][0:1, c * 128:(c + 1) * 128], d_r[:],
           start=True, stop=False, skip_group_check=True)
        mm(dpf[:, 0:256], C["c_b2row"][0:1, c * 128:(c + 1) * 128],
           C["c_onesrow"][0:1, 0:256], start=False, stop=True,
           skip_group_check=True)
        dgp.append(dpf)
    outfm = []
    for c in range(2):
        pt = sb.tile([128, W], BF16, tag=f"sq{c}")
        nc.vector.tensor_tensor(out=pt[:], in0=x2_sb[c][:], in1=c2p[:],
                                op=OP.mult)
        a1 = sb.tile([128, 256], BF16, tag=f"pa{c}")
        nc.vector.tensor_tensor(out=a1[:], in0=pt[:, 0:256],
                                in1=pt[:, 256:512], op=OP.add)
        nc.vector.tensor_tensor(out=a1[:], in0=a1[:], in1=pt[:, 512:768],
                                op=OP.add)
        of = pout.tile([128, 256], F32, tag=f"outfm{c}")
        nc.vector.scalar_tensor_tensor(
            out=of[:], in0=a1[:], scalar=C["c_g2pp"][:, c:c + 1],
            in1=dgp[c][:, 0:256], op0=OP.mult, op1=OP.add)
        outfm.append(of)
    for j in range(2):
        orp = psb.tile([128, W], BF16, tag="psB")
        orf = orp[:, 0:512].bitcast(F32)
        for c in range(2):
            mm(orf[:, c * 128:(c + 1) * 128],
               outfm[c][:, j * 128:(j + 1) * 128], idf,
               is_transpose=True, start=(c == 0), stop=(c == 1),
               skip_group_check=True)
        ot = pout.tile([128, 256], F32, tag=f"out{j}")
        nc.scalar.activation(ot[:], orf[:], AF.Copy)
        nc.sync.dma_start(OUT[B0 + j * 128:B0 + (j + 1) * 128, :], ot[:])


_CACHE: dict = {}


def _get_nc(bl: int) -> bass.Bass:
    if bl not in _CACHE:
        _CACHE[bl] = build(bl)
    return _CACHE[bl]


LAST_RESULT = None


def kernel(**inputs) -> np.ndarray:
    global LAST_RESULT
    consts = host_consts(inputs)
    nc = _get_nc(B // NCORES)
    in_maps = [host_inputs(i, inputs, consts) for i in range(NCORES)]
    res = run_bass_kernel_spmd(nc, in_maps, core_ids=list(range(NCORES)))
    LAST_RESULT = res
    return np.concatenate([r["out"] for r in res.results], axis=0)
